# revision 51
# baseline (speedup 1.0000x reference)
"""Causal multi-head attention on 8 Trainium2 NeuronCores.

Problem: B=4, S=2048, D=1024, H=16 heads, d_k=64, causal, fp32 in/out.

Sharding (host side): core c handles batch b=c//2 and head-half hh=c%2
(8 heads = 512 of the 1024 model dims). Each core computes its batch's
attention output for its 8 heads and the partial out-projection through
the matching 512 rows of Wo (+ bo/2, so the pair sums to +bo). The host
gathers by summing the two partials per batch. No collectives needed.

Final design (560us initial -> 369us -> ~295us):
 - all-bf16 PE path, PSUM accumulates fp32, exact causal trims.
   fp8-e4m3 DoubleRow QK-proj was measured at 1.06e-2 rel err (passes
   the 2e-2 gate) and -27us of PE stream, but NETS SLOWER (~298-306us):
   the removed proj work was doing double duty as anti-HAM filler and
   the late zone re-throttles (HAM drops PE to 1.2GHz after an idle-ish
   3.4us window; needs a fully-busy window to recover).
 - task order (0,*),(1,*),(3,*),(2,*) with per-task FILLER QUOTAS woven
   BETWEEN the kb-groups of each attention task: proj units deferred as
   late as deps allow (qk(c,dc) just-in-time per dc; note (3,dc) reads
   kt[dc] chunks 0..3, so qk(2,dc) must precede it), outproj units fill
   the ACT-bound (3,*)/(2,*) windows, tail = pure-PE outproj(2). This
   keeps the HAM at 2.4GHz for a ~240us stretch (was 43-80us of k=4/8
   clock-gating with coarser interleaves).
 - dummy standalone-LDWEIGHTS "warm" bursts were tried for the residual
   ~19us of k=4: they displace real work (the tile scheduler places by
   model-time readiness and hoists them) - net loss, removed.
 - PSUM: scores 2x[128,1024] (4 banks) + av 2x[65,512] + shared
   proj/outproj ring 2x[128,512] = 8 banks exactly
 - DMA: one transfer per tensor, split/ordered by first need (xt cols
   0:128 first so the first proj matmul starts ~13us; SP issue is
   ~700ns each and the preamble is ~7us, so the start is DMA-gated).
   Issuing DMAs from the ACT queue or 2-byte-element gpsimd broadcast
   DMAs both measured catastrophically slow - avoid.
 - output bf16 (halves writeback; host pair-sum in fp32 keeps the
   partial-sum error at ~4e-3 overall)
 - softmax reciprocal via reciprocal_approx_fast on an SBUF-staged
   copy (InstReciprocal on [1,512] costs 3.35us; approx is ~5x faster;
   PSUM input to the custom-DVE op silently returns garbage)

On-core layout:
  xT  [1024, 2048]  x[b]^T  bf16                  (host-transposed)
  Q^T, K^T [512, 2048] as 4 tiles [128, 2048]     (head pair per tile)
  V   16 tiles [128 keys, 8 heads x 65] bf16      (65th col = ones -> rowsums)
  scores S^T[k, q] = K^T.T @ Q^T  (contraction d=64; head A at partitions
        0-63, head B at 64-127 -> disjoint PE row groups)
  P^T = exp(0.125 * (S^T + causal mask)) on ACT, straight from PSUM
  AV: out^T[65, q] += V_ext[kb].T @ P^T[kb]  (k-blocks, causally trimmed)
  normalize: row 64 = rowsum -> reciprocal_approx_fast -> gpsimd
        partition_broadcast -> multiply into A^T tiles
  out[s, dm] = A^T.T @ Wo_local + bo/2
"""
import sys

for _p in ("/opt/trn_rl_repo",):
    if _p not in sys.path:
        sys.path.insert(0, _p)

import numpy as np

import concourse.bass as bass
import concourse.tile as tile
from concourse import bacc, bass_utils, library_config, mybir

F32 = mybir.dt.float32
BF16 = mybir.dt.bfloat16
EXPF = mybir.ActivationFunctionType.Exp
ADD = mybir.AluOpType.add
MULT = mybir.AluOpType.mult

D = 1024          # model dim
S = 2048          # sequence length
DL = 512          # local head dims (8 heads x 64)
NH = 8            # local heads
NC_ = 8           # cores
NEG = -1.0e30

_CACHE = {}
TRACE = False
last_results = None


def build_program():
    nc = bacc.Bacc("TRN2", target_bir_lowering=False, debug=False)

    # inputs pre-chunked on host: [p, c, n] = full[128c + p, n] so each
    # tensor loads with ONE big DMA (SP descriptor issue is ~640ns each;
    # 57 small DMAs cost ~35us of issue serialization)
    xt_d = nc.dram_tensor("xt", [128, 8, S], BF16, kind="ExternalInput").ap()
    wq_d = nc.dram_tensor("wq", [128, 8, DL], BF16, kind="ExternalInput").ap()
    wk_d = nc.dram_tensor("wk", [128, 8, DL], BF16, kind="ExternalInput").ap()
    wv_d = nc.dram_tensor("wv", [128, 8, DL], BF16, kind="ExternalInput").ap()
    wo_d = nc.dram_tensor("wo", [128, 4, D], BF16, kind="ExternalInput").ap()
    bq_d = nc.dram_tensor("bq2", [128, 4], F32, kind="ExternalInput").ap()
    bk_d = nc.dram_tensor("bk2", [128, 4], F32, kind="ExternalInput").ap()
    bv_d = nc.dram_tensor("bv", [DL], F32, kind="ExternalInput").ap()
    bo_d = nc.dram_tensor("boh", [D], F32, kind="ExternalInput").ap()
    tri_d = nc.dram_tensor("tri", [128, 128], F32, kind="ExternalInput").ap()
    out_d = nc.dram_tensor("out", [S, D], BF16, kind="ExternalOutput").ap()

    with tile.TileContext(nc) as tc:
        nc.gpsimd.load_library(library_config.attn)

        consts = tc.alloc_tile_pool(name="consts", bufs=1)

        # ---- constants (DMA starts for tri/bq2/bk2 are issued inside the
        # input block below so they don't delay the xt/wq/wk issue) ----
        tri = consts.tile([128, 128], F32, tag="tri", name="tri")
        bq2 = consts.tile([128, 4], F32, tag="bq2", name="bq2")
        bk2 = consts.tile([128, 4], F32, tag="bk2", name="bk2")
        bvb = consts.tile([128, DL], F32, tag="bvb", name="bvb")
        nc.gpsimd.dma_start(
            bvb,
            bass.AP(tensor=bv_d.tensor, offset=bv_d.offset,
                    ap=[[0, 128]] + bv_d.ap))
        bob = consts.tile([128, D], F32, tag="bob", name="bob")
        nc.gpsimd.dma_start(
            bob,
            bass.AP(tensor=bo_d.tensor, offset=bo_d.offset,
                    ap=[[0, 128]] + bo_d.ap))
        ones8 = consts.tile([128, NH], F32, tag="ones8", name="ones8")
        nc.vector.memset(ones8[:], 1.0)
        wseed = consts.tile([128, 128], BF16, tag="wseed", name="wseed")
        nc.vector.memset(wseed[:], 1.0)

        # ---- persistent data pools (all live together; no phase bars) --
        xtp = tc.alloc_tile_pool(name="xtp", bufs=1)
        xtall = xtp.tile([128, 8, S], BF16, tag="xt", name="xt")
        xt = [xtall[:, i, :] for i in range(8)]
        wqp = tc.alloc_tile_pool(name="wqp", bufs=1)
        wqall = wqp.tile([128, 8, DL], BF16, tag="wq", name="wq")
        wqt = [wqall[:, i, :] for i in range(8)]
        wkp = tc.alloc_tile_pool(name="wkp", bufs=1)
        wkall = wkp.tile([128, 8, DL], BF16, tag="wk", name="wk")
        wkt = [wkall[:, i, :] for i in range(8)]
        wvp = tc.alloc_tile_pool(name="wvp", bufs=1)
        wvall = wvp.tile([128, 8, DL], BF16, tag="wv", name="wv")
        wvt = [wvall[:, i, :] for i in range(8)]
        qkp = tc.alloc_tile_pool(name="qkp", bufs=1)
        qt = [qkp.tile([128, S], BF16, tag=f"qt{i}", name=f"qt{i}")
              for i in range(4)]
        kt = [qkp.tile([128, S], BF16, tag=f"kt{i}", name=f"kt{i}")
              for i in range(4)]
        vp = tc.alloc_tile_pool(name="vp", bufs=1)
        v = [vp.tile([128, NH, 65], BF16, tag=f"v{i}", name=f"v{i}")
             for i in range(16)]
        atp = tc.alloc_tile_pool(name="atp", bufs=1)
        at = [atp.tile([128, S], BF16, tag=f"at{i}", name=f"at{i}")
              for i in range(4)]
        wop = tc.alloc_tile_pool(name="wop", bufs=1)
        woall = wop.tile([128, 4, D], BF16, tag="wo", name="wo")
        wo = [woall[:, i, :] for i in range(4)]
        ptp = tc.alloc_tile_pool(name="ptp", bufs=24)
        rcp = tc.alloc_tile_pool(name="rcp", bufs=2)
        bcp = tc.alloc_tile_pool(name="bcp", bufs=2)
        outp = tc.alloc_tile_pool(name="outp", bufs=3)

        # PSUM: s4p 2x2 banks + avp 2x1 + auxp 2x1 = 8 banks. proj and
        # outproj share auxp, but outproj(j-1) is emitted AFTER
        # chunkproj(j+1) so ring predecessors are always ready-to-drain.
        s4p = tc.alloc_tile_pool(name="s4p", bufs=2, space="PSUM")
        avp = tc.alloc_tile_pool(name="avp", bufs=2, space="PSUM")
        auxp = tc.alloc_tile_pool(name="auxp", bufs=2, space="PSUM")

        # ---- input DMAs: task-(0,0) deps first (xt cols 0:256+256:512 +
        # the dc=0 slices of Wq/Wk), then the rest in need order. xt is
        # split so the first proj matmuls can start on the first half ----
        nc.sync.dma_start(xtall[:, :, 0:128], xt_d[:, :, 0:128])
        nc.sync.dma_start(wqall[:, :, 0:128], wq_d[:, :, 0:128])
        nc.sync.dma_start(wkall[:, :, 0:128], wk_d[:, :, 0:128])
        nc.sync.dma_start(xtall[:, :, 128:512], xt_d[:, :, 128:512])
        nc.sync.dma_start(bq2, bq_d)
        nc.sync.dma_start(bk2, bk_d)
        nc.sync.dma_start(tri, tri_d)
        nc.sync.dma_start(wqall[:, :, 128:DL], wq_d[:, :, 128:DL])
        nc.sync.dma_start(wkall[:, :, 128:DL], wk_d[:, :, 128:DL])
        nc.sync.dma_start(xtall[:, :, 512:1024], xt_d[:, :, 512:1024])
        nc.sync.dma_start(wvall[:], wv_d)
        nc.sync.dma_start(xtall[:, :, 1024:S], xt_d[:, :, 1024:S])
        nc.sync.dma_start(woall[:], wo_d)

        bvb3 = bvb[:].rearrange("p (h d) -> p h d", h=NH)

        # ================= emitters ==================================
        def emit_qk_proj(j, dc, split=False):
            # split=True: moving in two 256-wide pieces so the first
            # matmuls start as soon as the first xt DMA slice lands
            for wts, b2, dst in ((wqt, bq2, qt), (wkt, bk2, kt)):
                ps = auxp.tile([128, 512], F32, tag="aux", name="psqk")
                spans = ((0, 128), (128, 512)) if split else ((0, 512),)
                for lo, hi in spans:
                    for c in range(8):
                        nc.tensor.matmul(
                            ps[:, lo:hi],
                            wts[c][:, dc * 128:(dc + 1) * 128],
                            xt[c][:, j * 512 + lo:j * 512 + hi],
                            start=(c == 0), stop=(c == 7),
                            skip_group_check=(lo > 0))
                nc.vector.tensor_scalar_add(
                    dst[dc][:, j * 512:(j + 1) * 512],
                    ps[:], b2[:, dc:dc + 1])

        def emit_v_proj(sb):
            ps = auxp.tile([128, 512], F32, tag="aux", name="psv")
            for c in range(8):
                nc.tensor.matmul(
                    ps[:],
                    xt[c][:, sb * 128:(sb + 1) * 128],
                    wvt[c][:],
                    start=(c == 0), stop=(c == 7))
            nc.vector.tensor_tensor(
                v[sb][:, :, 0:64],
                ps[:].rearrange("p (h d) -> p h d", h=NH),
                bvb3, op=ADD)
            nc.vector.tensor_copy(v[sb][:, :, 64], ones8[:])

        def emit_outproj_sb(sb):
            ot = outp.tile([128, D], BF16, tag="ot", name="ot")
            for n in range(2):
                ps = auxp.tile([128, 512], F32, tag="aux", name="psd")
                for hc in range(4):
                    nc.tensor.matmul(
                        ps[:],
                        at[hc][:, sb * 128:(sb + 1) * 128],
                        wo[hc][:, n * 512:(n + 1) * 512],
                        start=(hc == 0), stop=(hc == 3))
                nc.vector.tensor_tensor(
                    ot[:, n * 512:(n + 1) * 512], ps[:],
                    bob[:, n * 512:(n + 1) * 512], op=ADD)
            nc.sync.dma_start(out_d[sb * 128:(sb + 1) * 128, :], ot[:])

        def warm(n, anchor):
            # Dummy standalone LDWEIGHTS: ~95ns of pure PE busy-work, no
            # PSUM, no consumers. Used where the PE has a GENUINE wait (the
            # startup DMA stall, the DMA-thin (0,*) zone): the HAM watches a
            # free-running 3.4us activity window and keeps the PE at 1.2GHz
            # until it sees a busy one, so idle-filling dummies move the
            # 2.4GHz onset earlier (measured: k=8 onset 22.6us -> 17.2us).
            # The anchor read-dep pins placement (the tile scheduler places
            # by model-time readiness).
            for i in range(n):
                nc.tensor.ldweights(
                    anchor[0:64, 32 * (i % 4):32 * (i % 4) + 32])

        def emit_pass1(dc, j, filler=(), warm_n=0):
            """Scores + exp for all k-blocks of q-chunk j; returns pt tiles.

            filler: list of zero-arg emitters (proj/outproj units) woven in
            after each kb-group so the PE list scheduler always has dense
            independent matmul work between the dep-chained attention ops
            (keeps the HAM activity window busy -> clock stays at 2.4GHz).
            """
            filler = list(filler)
            nf = 0
            ng = 2 * j + 2
            pts = {}              # (g, half) -> pt tile (bf16)
            for g in range(ng):   # kb-groups of 2
                s4s = {}
                cs_list = []
                for kk in range(2):
                    kb = 2 * g + kk
                    cs = max(0, 128 * kb - 512 * j)   # exact causal trim
                    cs_list.append((kb, cs))
                # scores: interleave halves so A (rows 0-63) and B
                # (rows 64-127) can overlap in disjoint PE row groups
                for half in range(2):
                    s4s[half] = s4p.tile([128, 1024], F32,
                                         tag="s4", name="s4")
                for kk, (kb, cs) in enumerate(cs_list):
                    for half in range(2):
                        pr = 64 * half
                        nc.tensor.matmul(
                            s4s[half][:, 512 * kk + cs:512 * (kk + 1)],
                            kt[dc][pr:pr + 64, 128 * kb:128 * (kb + 1)],
                            qt[dc][pr:pr + 64, 512 * j + cs:512 * (j + 1)],
                            start=True, stop=True)
                for half in range(2):
                    s4 = s4s[half]
                    for kk, (kb, cs) in enumerate(cs_list):
                        if 128 * kb >= 512 * j:     # diagonal block
                            sl = s4[:, 512 * kk + cs:512 * kk + cs + 128]
                            nc.vector.tensor_tensor(sl, sl, tri[:], op=ADD)
                    cs0 = cs_list[0][1]
                    cs1 = cs_list[1][1]
                    pt = ptp.tile([128, 1024], BF16, tag="pt", name="pt")
                    pts[(g, half)] = pt
                    if cs1 < 352:
                        # merged call; [512:512+cs1) is never-read garbage
                        nc.scalar.activation(
                            pt[:, cs0:1024], s4[:, cs0:1024],
                            EXPF, scale=0.125)
                    else:
                        nc.scalar.activation(
                            pt[:, cs0:512], s4[:, cs0:512],
                            EXPF, scale=0.125)
                        nc.scalar.activation(
                            pt[:, 512 + cs1:1024], s4[:, 512 + cs1:1024],
                            EXPF, scale=0.125)
                # weave filler units evenly across the kb-groups
                while nf < len(filler) * (g + 1) // ng:
                    filler[nf]()
                    nf += 1
                if warm_n:
                    warm(warm_n, pts[(g, 0)])
            return pts

        def emit_pass2(dc, j, pts):
            """One long AV accumulation chain per head + normalize."""
            for half in range(2):
                pr = 64 * half
                av = avp.tile([65, 512], F32, tag="av", name="av")
                for g in range(2 * j + 2):
                    pt = pts[(g, half)]
                    for kk in range(2):
                        kb = 2 * g + kk
                        cs = max(0, 128 * kb - 512 * j)
                        first = (g == 0 and kk == 0)
                        nc.tensor.matmul(
                            av[:, cs:512],
                            v[kb][:, 2 * dc + half, :],
                            pt[:, 512 * kk + cs:512 * (kk + 1)],
                            start=first, stop=True,
                            skip_group_check=not first)
                rsum = rcp.tile([1, 512], F32, tag="rsum", name="rsum")
                nc.vector.tensor_copy(rsum[:], av[64:65, :])
                rec = rcp.tile([1, 512], F32, tag="rec", name="rec")
                nc.vector.reciprocal_approx_fast(rec[:], rsum[:])
                bc = bcp.tile([64, 512], F32, tag="bc", name="bc")
                nc.gpsimd.partition_broadcast(bc[:], rec[:])
                nc.vector.tensor_tensor(
                    at[dc][pr:pr + 64, 512 * j:512 * (j + 1)],
                    av[0:64, :], bc[:], op=MULT)

        # ================= schedule ==================================
        # Fine-grained interleave: attention tasks carry a per-task quota
        # of independent PE filler (proj/outproj units) woven BETWEEN the
        # kb-groups of pass1, sized so ACT-heavy tasks never leave the PE
        # with sub-window gaps (HAM drops the clock to 1.2GHz after one
        # idle-ish 3.4us window and needs a fully-busy one to recover).
        # Chunk order ends (3,3),(2,3) with the oj2+oj3 pure-PE tail.
        # Deps: qk/v units of chunk c precede any (c,*) task; oj units of
        # chunk c follow pass2(c,*) (pass2 of task T is emitted during
        # the task after T).
        def u_qk(c, dcx):
            return lambda: emit_qk_proj(c, dcx)

        def u_v(sb):
            return lambda: emit_v_proj(sb)

        def u_oj(sb):
            return lambda: emit_outproj_sb(sb)

        # Filler is deferred as late as deps allow: qk(2,dc) is needed only
        # before task (2,dc) -> it fills the ACT-bound (3,*) window; v8-15
        # only before pass2(3,0) (emitted during (3,1)); outproj units fill
        # the (2,*) window. This keeps the PE dense in the late zone where
        # ACT otherwise outpaces it.
        # warm the HAM during the startup DMA stall: PE is idle 7-13us
        # waiting for the first xt/wq slices; these dummies run in that
        # window (anchored on the memset wseed, ready at ~6.5us)
        warm(36, wseed)
        emit_qk_proj(0, 0, split=True)
        # NOTE: task (3,dc) reads kt[dc]/qt[dc] for key chunks 0..3, so BOTH
        # qk(2,dc) and qk(3,dc) must be emitted before pass1(3,dc) — the
        # qk deferral is per-dc just-in-time (qk(2,1)/qk(3,1) during (3,0),
        # etc.), never woven into the task that consumes them.
        plan = [
            ((0, 0), [], 0),                              # c0 rest special-cased
            ((0, 1), [u_qk(1, 0), u_qk(1, 1)], 3),
            ((0, 2), [u_qk(1, 2), u_qk(1, 3)], 3),
            ((0, 3), [u_v(4), u_v(5), u_v(6), u_v(7)], 3),
            ((1, 0), [u_v(8)], 3),
            ((1, 1), [u_v(9), u_v(10)], 0),
            ((1, 2), [u_v(11), u_qk(2, 0)], 0),
            ((1, 3), [u_qk(3, 0), u_v(12)], 0),
            ((3, 0), [u_qk(2, 1), u_qk(3, 1), u_v(13), u_v(14), u_v(15)], 0),
            ((3, 1), [u_qk(2, 2), u_qk(3, 2)], 0),
            ((3, 2), [u_qk(2, 3), u_qk(3, 3)], 0),
            ((3, 3), [u_oj(0), u_oj(1)], 0),
            ((2, 0), [u_oj(2), u_oj(3)], 0),
            ((2, 1), [u_oj(4), u_oj(5)], 0),
            ((2, 2), [u_oj(6), u_oj(7), u_oj(12)], 0),
            ((2, 3), [u_oj(13), u_oj(14)], 0),
        ]
        prev = None
        for (j, dc), filler, wn in plan:
            pts = emit_pass1(dc, j, filler, warm_n=wn)
            if (j, dc) == (0, 0):
                # rest of chunk-0 projections run during task (0,0);
                # QK(0,1) first so scores(0,1) unblocks soonest
                emit_qk_proj(0, 1)
                for sb in range(4):
                    emit_v_proj(sb)
                emit_qk_proj(0, 2)
                emit_qk_proj(0, 3)
            if prev is not None:
                emit_pass2(*prev)
            prev = (dc, j, pts)
        emit_pass2(*prev)
        for sb in (15, 8, 9, 10, 11):
            emit_outproj_sb(sb)

        auxp.release()
        avp.release()
        s4p.release()
        outp.release()
        bcp.release()
        rcp.release()
        ptp.release()
        wop.release()
        atp.release()
        vp.release()
        qkp.release()
        wvp.release()
        wkp.release()
        wqp.release()
        xtp.release()
        consts.release()

    nc.compile()
    return nc


def make_in_maps(x, Wq, bq, Wk, bk, Wv, bv, Wo, bo):
    from ml_dtypes import bfloat16
    x = np.asarray(x, np.float32)
    Wq, bq = np.asarray(Wq, np.float32), np.asarray(bq, np.float32)
    Wk, bk = np.asarray(Wk, np.float32), np.asarray(bk, np.float32)
    Wv, bv = np.asarray(Wv, np.float32), np.asarray(bv, np.float32)
    Wo, bo = np.asarray(Wo, np.float32), np.asarray(bo, np.float32)

    k = np.arange(128)[:, None]
    c = np.arange(128)[None, :]
    tri = np.where(c >= k, 0.0, NEG).astype(np.float32)
    boh = (bo * 0.5).astype(np.float32)

    def chunked(a):
        """[128*nc, n] -> [128, nc, n] bf16 with [p, c, n] = a[128c+p, n]."""
        nch = a.shape[0] // 128
        return np.ascontiguousarray(
            a.reshape(nch, 128, a.shape[1]).transpose(1, 0, 2).astype(bfloat16))

    in_maps = []
    for core in range(NC_):
        b, hh = core // 2, core % 2
        sl = slice(hh * DL, (hh + 1) * DL)
        in_maps.append({
            "xt": chunked(x[b].T),
            "wq": chunked(Wq[:, sl]),
            "wk": chunked(Wk[:, sl]),
            "wv": chunked(Wv[:, sl]),
            "wo": chunked(Wo[sl, :]),
            "bq2": np.ascontiguousarray(bq[sl].reshape(4, 128).T),
            "bk2": np.ascontiguousarray(bk[sl].reshape(4, 128).T),
            "bv": np.ascontiguousarray(bv[sl]),
            "boh": boh,
            "tri": tri,
        })
    return in_maps


def kernel(x, Wq, bq, Wk, bk, Wv, bv, Wo, bo):
    global last_results
    if "nc" not in _CACHE:
        _CACHE["nc"] = build_program()
    nc = _CACHE["nc"]
    in_maps = make_in_maps(x, Wq, bq, Wk, bk, Wv, bv, Wo, bo)
    res = bass_utils.run_bass_kernel_spmd(
        nc, in_maps, core_ids=list(range(NC_)), trace=TRACE)
    last_results = res
    B = 4
    out = np.empty((B, S, D), np.float32)
    for b in range(B):
        out[b] = (res.results[2 * b]["out"].astype(np.float32)
                  + res.results[2 * b + 1]["out"].astype(np.float32))
    return out

